# revision 3
# baseline (speedup 1.0000x reference)
"""EdgeNetworkLayer Trainium2 kernel v3: target-sharded, fine-grained pipeline.

v2 -> v3 changes (trace-driven):
- Per-tile gather buffers + per-half hwT/H32 + per-chunk mT so consumers wait
  on exactly the producers they need (v2's single tiles made the first
  transpose wait for all 16 gathers etc).
- Z32 replicated z tiles prebuilt ONCE for the full edge range (16 tiles, 64
  broadcast DMAs in the startup window) instead of 128 per-(half,g) DMAs
  whose ~0.6us issue cost clogged the sync/scalar queues during the main loop.
- Z phase (edge MLP layer 1) in fp16 instead of fp32 (2-pass fp32 matmuls
  cost 8.5us of early PE time in v2).
- h gathered in fp16 (h16 input) -> 1-cycle fp16 transposes.
- PT product split: DVE builds b-slices 0-2, Pool (gpsimd) builds b-slice 3;
  DVE was the main-loop pole at ~2.29us/group vs PE ~1.9us.
- GRU runs in 256-col chunks; chunk 0 (+1) overlap the second edge half via
  a shared PSUM budget (acc bufs=1: 2 + pst2 1 + psm 1 + psg 3 = 7 banks).
"""
import numpy as np

N, H, E, ED, MLP_HID = 8192, 128, 16384, 16, 64
NCORES = 8
P = 128
NS = N // NCORES          # 1024 nodes per core
NST = NS // P             # 8 local node groups
KG = 16                   # k-groups of 4
GCH = 256                 # GRU column chunk

EXACT_FP32 = False


def _host_prep(h, edge_index, edge_features, W1, b1, W2, b2, W_ih, W_hh, b_ih, b_hh):
    f32 = np.float32
    f16 = np.float16
    sdt = f32 if EXACT_FP32 else f16
    h = np.ascontiguousarray(h, f32)
    src_all = np.asarray(edge_index[0], np.int64)
    tgt_all = np.asarray(edge_index[1], np.int64)
    ef_all = np.asarray(edge_features, f32)

    # LPT node->core assignment balancing edge counts (cap NS nodes/core)
    deg = np.bincount(tgt_all, minlength=N)
    order = np.argsort(-deg, kind="stable")
    loads = np.zeros(NCORES, np.int64)
    ncnt = np.zeros(NCORES, np.int64)
    assign = np.zeros(N, np.int64)
    for v in order:
        best, bl = -1, None
        for c in range(NCORES):
            if ncnt[c] < NS and (bl is None or loads[c] < bl):
                best, bl = c, loads[c]
        assign[v] = best
        loads[best] += deg[v]
        ncnt[best] += 1

    node_lists = [np.where(assign == c)[0] for c in range(NCORES)]
    local = np.zeros(N, np.int64)
    for c in range(NCORES):
        local[node_lists[c]] = np.arange(NS)

    shards = []
    for c in range(NCORES):
        m = assign[tgt_all] == c
        s, t, ef = src_all[m], local[tgt_all[m]], ef_all[m]
        o = np.argsort(t, kind="stable")
        shards.append((s[o], t[o], np.ascontiguousarray(ef[o])))
    cnt = [len(s) for s, _, _ in shards]
    ETP = (max(cnt) + P - 1) // P
    ESP = ETP * P

    # uniform band plan across cores (real edges only)
    base = np.full(ETP, NS, np.int64)
    endv = np.zeros(ETP, np.int64)
    for ti in range(ETP):
        for c in range(NCORES):
            t = shards[c][1]
            lo, hi = ti * P, min((ti + 1) * P, cnt[c])
            if lo >= hi:
                continue
            base[ti] = min(base[ti], (int(t[lo]) // P) * P)
            endv[ti] = max(endv[ti], int(t[hi - 1]) + 1)
    W_band = int(np.max(endv - base))
    W_band = max(P, ((W_band + P - 1) // P) * P)
    W_band = min(W_band, NS)
    base = np.maximum(np.minimum(base, NS - W_band), 0)

    # exact contributors: union across cores of the node groups each edge
    # tile actually targets (band coverage over-approximates ~2x)
    touched = [set() for _ in range(ETP)]
    for c in range(NCORES):
        t = shards[c][1]
        for ti in range(ETP):
            lo, hi = ti * P, min((ti + 1) * P, cnt[c])
            if lo < hi:
                touched[ti].update(np.unique(t[lo:hi] // P).tolist())
    contrib = [[] for _ in range(NST)]
    for ti in range(ETP):
        for ng in sorted(touched[ti]):
            contrib[ng].append(ti)

    # W2 tiles: [(g,b), (a,c), i]; host layout [p, t, i]
    W2r = np.asarray(W2, f32).reshape(MLP_HID, H, H)            # [k, i, j]
    W2g = W2r.reshape(KG, 4, H, 4, 32)                          # [g, a, i, b, c]
    W2p = W2g.transpose(0, 3, 1, 4, 2).reshape(64, P, H)        # [(g,b), (a,c), i]
    wdt = f32 if EXACT_FP32 else f16
    W2P_host = np.ascontiguousarray(W2p.transpose(1, 0, 2).astype(wdt))  # [p, 64, i]
    W2P32_host = np.ascontiguousarray(
        np.asarray(b2, f32).reshape(H, H).T.astype(wdt))

    W1p = np.concatenate([np.asarray(W1, f32), np.asarray(b1, f32)[None, :]],
                         0).astype(wdt)

    W_ihT = np.ascontiguousarray(np.asarray(W_ih, f32).T.astype(wdt))   # [128, 384]
    W_hhT = np.ascontiguousarray(np.asarray(W_hh, f32).T.astype(wdt))
    b_ih = np.asarray(b_ih, f32)
    b_hh = np.asarray(b_hh, f32)
    b_r = (b_ih[:H] + b_hh[:H]).reshape(H, 1).astype(f32)
    b_z = (b_ih[H:2 * H] + b_hh[H:2 * H]).reshape(H, 1).astype(f32)
    b_in = b_ih[2 * H:].reshape(H, 1).astype(f32)
    b_hn = b_hh[2 * H:].reshape(H, 1).astype(f32)

    in_maps = []
    for c in range(NCORES):
        s, t, ef = shards[c]
        n = cnt[c]
        efT = np.zeros((ED + 1, ESP), wdt)
        efT[:ED, :n] = ef.T
        efT[ED, :n] = 1.0                                       # b1 ones-row
        srcidx = np.zeros(ESP, np.int32)
        srcidx[:n] = s
        srcidx = np.ascontiguousarray(srcidx.reshape(ETP, P).T)  # [128, ETP]
        Sband = np.zeros((ETP, P, W_band), sdt)
        for ti in range(ETP):
            lo, hi = ti * P, min((ti + 1) * P, n)
            for r in range(lo, hi):
                Sband[ti, r - lo, int(t[r]) - int(base[ti])] = 1.0
        Sband = np.ascontiguousarray(Sband.transpose(1, 0, 2))   # [128, ETP, W]
        hTs = np.ascontiguousarray(h[node_lists[c]].T)           # [128, 1024]
        in_maps.append(dict(
            h16=h.astype(wdt), efT=efT, srcidx=srcidx, Sband=Sband,
            W2P=W2P_host, W2P32=W2P32_host, W1p=W1p, WihT=W_ihT, WhhT=W_hhT,
            b_r=b_r, b_z=b_z, b_in=b_in, b_hn=b_hn, hTs=hTs))
    plan = (ETP, W_band, tuple(int(b) for b in base),
            tuple(tuple(c_) for c_ in contrib))
    return in_maps, node_lists, plan


def _build_program(ETP, W_band, base, contrib):
    import concourse.bass as bass
    import concourse.bacc as bacc
    import concourse.tile as tile
    import concourse.mybir as mybir
    from concourse.masks import make_identity

    dt = mybir.dt.float32
    f16 = mybir.dt.float16
    dtr = dt if EXACT_FP32 else f16    # main matmul operand dtype
    dts = dt if EXACT_FP32 else f16    # scatter dtype
    dtg = dt if EXACT_FP32 else f16    # GRU matmul operand dtype
    dti = mybir.dt.int32
    AF = mybir.ActivationFunctionType
    OP = mybir.AluOpType

    ESP = ETP * P
    HT0 = (ETP + 1) // 2      # tiles in half 0
    HT1 = ETP - HT0
    EH0, EH1 = HT0 * P, HT1 * P
    HTS = [HT0, HT1]
    EHS = [EH0, EH1]
    EOFF = [0, EH0]
    groupsA = [ng for ng in range(NST)
               if contrib[ng] and max(contrib[ng]) < HT0]
    groupsB = [ng for ng in range(NST) if ng not in groupsA]
    # GRU chunks whose node groups are all scatterable after half 0
    NCH = NS // GCH
    gpc = GCH // P            # node groups per GRU chunk
    chunksA = [ci for ci in range(NCH)
               if all(ng in groupsA for ng in range(ci * gpc, (ci + 1) * gpc))]
    chunksB = [ci for ci in range(NCH) if ci not in chunksA]

    def chunks(total, step=512):
        out, c0 = [], 0
        while c0 < total:
            out.append((c0, min(step, total - c0)))
            c0 += step
        return out

    nc = bacc.Bacc("TRN2", target_bir_lowering=False, debug=False,
                   num_devices=NCORES)

    h_d = nc.dram_tensor("h16", [N, H], dtr, kind="ExternalInput")
    efT_d = nc.dram_tensor("efT", [ED + 1, ESP], dtr, kind="ExternalInput")
    src_d = nc.dram_tensor("srcidx", [P, ETP], dti, kind="ExternalInput")
    S_d = nc.dram_tensor("Sband", [P, ETP, W_band], dts, kind="ExternalInput")
    W2P_d = nc.dram_tensor("W2P", [P, 64, H], dtr, kind="ExternalInput")
    W2P32_d = nc.dram_tensor("W2P32", [P, H], dtr, kind="ExternalInput")
    W1p_d = nc.dram_tensor("W1p", [ED + 1, MLP_HID], dtr, kind="ExternalInput")
    WihT_d = nc.dram_tensor("WihT", [H, 3 * H], dtg, kind="ExternalInput")
    WhhT_d = nc.dram_tensor("WhhT", [H, 3 * H], dtg, kind="ExternalInput")
    br_d = nc.dram_tensor("b_r", [H, 1], dt, kind="ExternalInput")
    bz_d = nc.dram_tensor("b_z", [H, 1], dt, kind="ExternalInput")
    bin_d = nc.dram_tensor("b_in", [H, 1], dt, kind="ExternalInput")
    bhn_d = nc.dram_tensor("b_hn", [H, 1], dt, kind="ExternalInput")
    hTs_d = nc.dram_tensor("hTs", [H, NS], dt, kind="ExternalInput")
    out_d = nc.dram_tensor("out_hT", [H, NS], dt, kind="ExternalOutput")

    with tile.TileContext(nc) as tc:
        with (
            tc.tile_pool(name="const", bufs=1) as cp,
            tc.tile_pool(name="dram", bufs=1, space="DRAM") as dram,
            tc.tile_pool(name="work", bufs=1) as wp,
        ):
            # ---------- gathers first (they head the critical chain)
            srci = cp.tile([P, ETP], dti)
            nc.sync.dma_start(srci[:], src_d[:])
            hw_t = []
            for t in range(ETP):
                hwt = wp.tile([P, P], dtr, tag=f"hw{t}")
                nc.gpsimd.indirect_dma_start(
                    out=hwt[:], out_offset=None, in_=h_d[:],
                    in_offset=bass.IndirectOffsetOnAxis(ap=srci[:, t:t + 1], axis=0))
                hw_t.append(hwt)

            ident = cp.tile([P, P], dt)
            make_identity(nc, ident[:])
            idf16 = cp.tile([P, P], dtr)
            nc.vector.tensor_copy(idf16[:], ident[:])
            efT = cp.tile([ED + 1, ESP], dtr)
            nc.sync.dma_start(efT[:], efT_d[:])
            W1p = cp.tile([ED + 1, MLP_HID], dtr)
            nc.sync.dma_start(W1p[:], W1p_d[:])

            # W2 resident in SBUF (2MB fp16) on scalar queue (idle at start)
            W2S = cp.tile([P, 64, H], dtr)
            nc.scalar.dma_start(W2S[:], W2P_d[:])
            w2t32 = cp.tile([P, H], dtr)
            nc.scalar.dma_start(w2t32[:], W2P32_d[:])

            zT_dram = dram.tile([MLP_HID, ESP], dtr)

            # ---------- phase Z (fp16) + h_w transposes (fp16), per half
            hwT_h = [wp.tile([P, EH0], dtr, tag="hwT0", name="hwT0"),
                     wp.tile([P, EH1], dtr, tag="hwT1", name="hwT1")]
            H32_h = [wp.tile([P, 4, EH0], dtr, tag="H320", name="H320"),
                     wp.tile([P, 4, EH1], dtr, tag="H321", name="H321")]
            with (
                tc.tile_pool(name="psz", bufs=1, space="PSUM") as psz,
                tc.tile_pool(name="pst", bufs=3, space="PSUM") as pst,
            ):
                zps = psz.tile([MLP_HID, ESP], dt, tag="zps")
                for c0, cw in chunks(ESP):
                    nc.tensor.matmul(zps[:, c0:c0 + cw], W1p[:],
                                     efT[:, c0:c0 + cw], start=True, stop=True)
                zT = wp.tile([MLP_HID, ESP], dtr)
                nc.scalar.activation(zT[:], zps[:], AF.Relu)
                nc.sync.dma_start(zT_dram[:], zT[:])

                for hh in range(2):
                    hwT = hwT_h[hh]
                    for t in range(HTS[hh]):
                        tp = pst.tile([P, P], dtr, tag="tp")
                        nc.tensor.transpose(tp[:], hw_t[EOFF[hh] // P + t][:],
                                            idf16[:])
                        nc.scalar.copy(hwT[:, t * P:(t + 1) * P], tp[:])
                    H32 = H32_h[hh]
                    for b in range(4):
                        for a in range(4):
                            eng = (nc.scalar, nc.sync, nc.gpsimd)[(b * 4 + a) % 3]
                            eng.dma_start(
                                H32[32 * a:32 * a + 32, b, :],
                                hwT[32 * b:32 * b + 32, :])

            # ---------- prebuilt replicated z tiles: Z32g[g][(a,c), e] = z[4g+a, e]
            Z32g = []
            for g in range(KG):
                zg = wp.tile([P, ESP], dtr, tag=f"z32g{g}")
                for a in range(4):
                    eng = (nc.sync, nc.scalar, nc.gpsimd)[(4 * g + a) % 3]
                    eng.dma_start(
                        zg[32 * a:32 * a + 32, :],
                        zT_dram[4 * g + a:4 * g + a + 1, :]
                        .broadcast_to((32, ESP)))
                Z32g.append(zg)

            # S tiles + GRU params on gpsimd queue (after gathers)
            s_tiles = {}
            with tc.tile_pool(name="spool", bufs=ETP) as spool:
                for ti in range(ETP):
                    stile = spool.tile([P, W_band], dts, tag="sel")
                    nc.gpsimd.dma_start(stile[:], S_d[:, ti, :])
                    s_tiles[ti] = stile
                WihT = cp.tile([H, 3 * H], dtg)
                nc.gpsimd.dma_start(WihT[:], WihT_d[:])
                WhhT = cp.tile([H, 3 * H], dtg)
                nc.gpsimd.dma_start(WhhT[:], WhhT_d[:])
                b_r = cp.tile([H, 1], dt)
                nc.gpsimd.dma_start(b_r[:], br_d[:])
                b_z = cp.tile([H, 1], dt)
                nc.gpsimd.dma_start(b_z[:], bz_d[:])
                b_in = cp.tile([H, 1], dt)
                nc.gpsimd.dma_start(b_in[:], bin_d[:])
                b_hn = cp.tile([H, 1], dt)
                nc.gpsimd.dma_start(b_hn[:], bhn_d[:])
                hTs = cp.tile([H, NS], dt)
                nc.gpsimd.dma_start(hTs[:], hTs_d[:])
                hTsg = cp.tile([H, NS], dtg)
                nc.scalar.copy(hTsg[:], hTs[:])

                # ---------- main + scatter + GRU, pipelined
                msgT_h = [wp.tile([P, EH0], dtr, tag="msgTa", name="msgTa"),
                          wp.tile([P, EH1], dtr, tag="msgTb", name="msgTb")]
                msg_h = [wp.tile([P, HT0, P], dts, tag="msga", name="msga"),
                         wp.tile([P, HT1, P], dts, tag="msgb", name="msgb")]
                mT_c = [wp.tile([H, GCH], dtg, tag=f"mT{ci}", name=f"mT{ci}")
                        for ci in range(NCH)]
                out_c = [wp.tile([H, GCH], dt, tag=f"out{ci}", name=f"out{ci}")
                         for ci in range(NCH)]

                def scatter_pass(ngl):
                    for ng in ngl:
                        cs = contrib[ng]
                        st = stage.tile([P, H], dts, tag="mstage")
                        if not cs:
                            nc.vector.memset(st[:], 0.0)
                        else:
                            pm = psm.tile([P, H], dt, tag="pm")
                            for idx, ti in enumerate(cs):
                                off = ng * P - int(base[ti])
                                half = 0 if ti < HT0 else 1
                                nc.tensor.matmul(
                                    pm[:], s_tiles[ti][:, off:off + P],
                                    msg_h[half][:, ti - EOFF[half] // P, :],
                                    start=(idx == 0), stop=(idx == len(cs) - 1))
                            nc.scalar.copy(st[:], pm[:])
                        tp = pst2.tile([P, P], dts, tag="tp2")
                        nc.tensor.transpose(tp[:], st[:], idf16[:])
                        mT = mT_c[ng // gpc]
                        nc.scalar.copy(
                            mT[:, (ng % gpc) * P:(ng % gpc + 1) * P], tp[:])

                def gru_chunk(ci):
                    mT = mT_c[ci]
                    osl = slice(ci * GCH, (ci + 1) * GCH)
                    cw = GCH
                    sfx = str(ci)
                    rz_ps = psg.tile([H, 2, GCH], dt, tag="rzp")
                    gin_ps = psg.tile([H, GCH], dt, tag="ginp")
                    ghn_ps = psg.tile([H, GCH], dt, tag="ghnp")
                    for q in range(2):
                        nc.tensor.matmul(rz_ps[:, q, :], WihT[:, q * H:(q + 1) * H],
                                         mT[:], start=True, stop=False)
                        nc.tensor.matmul(rz_ps[:, q, :], WhhT[:, q * H:(q + 1) * H],
                                         hTsg[:, osl], start=False, stop=True)
                    nc.tensor.matmul(gin_ps[:], WihT[:, 2 * H:3 * H],
                                     mT[:], start=True, stop=True)
                    nc.tensor.matmul(ghn_ps[:], WhhT[:, 2 * H:3 * H],
                                     hTsg[:, osl], start=True, stop=True)
                    rz = wp.tile([H, 2, GCH], dt, tag="rz" + sfx, name="rz" + sfx)
                    nc.scalar.activation(rz[:, 0, :], rz_ps[:, 0, :],
                                         AF.Sigmoid, bias=b_r[:])
                    nc.scalar.activation(rz[:, 1, :], rz_ps[:, 1, :],
                                         AF.Sigmoid, bias=b_z[:])
                    # n = tanh(gi_n + b_in + r*(gh_n + b_hn))
                    ghn = wp.tile([H, GCH], dt, tag="ghn" + sfx, name="ghn" + sfx)
                    nc.scalar.activation(ghn[:], ghn_ps[:], AF.Identity,
                                         bias=b_hn[:])
                    nc.vector.tensor_mul(ghn[:], rz[:, 0, :], ghn[:])
                    nc.vector.tensor_add(ghn[:], ghn[:], gin_ps[:])
                    ng_ = wp.tile([H, GCH], dt, tag="ng" + sfx, name="ng" + sfx)
                    nc.scalar.activation(ng_[:], ghn[:], AF.Tanh, bias=b_in[:])
                    # hnew = n + z*(h - n)
                    dif = wp.tile([H, GCH], dt, tag="dif" + sfx, name="dif" + sfx)
                    nc.vector.tensor_sub(dif[:], hTs[:, osl], ng_[:])
                    nc.vector.tensor_mul(dif[:], rz[:, 1, :], dif[:])
                    nc.vector.tensor_add(out_c[ci][:], ng_[:], dif[:])
                    nc.sync.dma_start(out_d[:, osl], out_c[ci][:])

                with (
                    tc.tile_pool(name="psacc", bufs=1, space="PSUM") as psacc,
                    tc.tile_pool(name="pst2", bufs=1, space="PSUM") as pst2,
                    tc.tile_pool(name="psm", bufs=1, space="PSUM") as psm,
                    tc.tile_pool(name="psg", bufs=1, space="PSUM") as psg,
                    tc.tile_pool(name="stage", bufs=4) as stage,
                    tc.tile_pool(name="ptpool", bufs=3) as ptpool,
                ):
                    for half in range(2):
                        EH = EHS[half]
                        esl = slice(EOFF[half], EOFF[half] + EH)
                        hwT = hwT_h[half]
                        H32 = H32_h[half]
                        acc = psacc.tile([P, EH0], dt, tag="acc")
                        for g in range(KG):
                            pt = ptpool.tile([P, 4, EH0], dtr, tag="pt")
                            nc.vector.tensor_tensor(
                                pt[:, :, :EH],
                                Z32g[g][:, esl].unsqueeze(1)
                                .broadcast_to((P, 4, EH)),
                                H32[:, :, :], OP.mult)
                            for b_ in range(4):
                                tw = 4 * g + b_
                                for c0, cw in chunks(EH):
                                    nc.tensor.matmul(
                                        acc[:, c0:c0 + cw],
                                        W2S[:, tw, :],
                                        pt[:, b_, c0:c0 + cw],
                                        start=(tw == 0), stop=False)
                        for c0, cw in chunks(EH):
                            nc.tensor.matmul(acc[:, c0:c0 + cw], w2t32[:],
                                             hwT[:, c0:c0 + cw],
                                             start=False, stop=(c0 + cw == EH))
                        msgT = msgT_h[half]
                        for c0, cw in chunks(EH):
                            nc.scalar.copy(msgT[:, c0:c0 + cw],
                                           acc[:, c0:c0 + cw])
                        msg = msg_h[half]
                        for t in range(HTS[half]):
                            tp = pst2.tile([P, P], dts, tag="tp2")
                            nc.tensor.transpose(tp[:],
                                                msgT[:, t * P:(t + 1) * P],
                                                idf16[:])
                            nc.scalar.copy(msg[:, t, :], tp[:])
                        if half == 0:
                            scatter_pass(groupsA)
                            for ci in chunksA:
                                gru_chunk(ci)
                    scatter_pass(groupsB)
                    for ci in chunksB:
                        gru_chunk(ci)

    nc.compile()
    return nc


_CACHE = {}


def _get_program(plan):
    if plan not in _CACHE:
        _CACHE[plan] = _build_program(*plan)
    return _CACHE[plan]


def kernel(h, edge_index, edge_features, W1, b1, W2, b2, W_ih, W_hh, b_ih, b_hh):
    from concourse import bass_utils

    in_maps, node_lists, plan = _host_prep(
        h, edge_index, edge_features, W1, b1, W2, b2, W_ih, W_hh, b_ih, b_hh)
    nc = _get_program(plan)
    res = bass_utils.run_bass_kernel_spmd(nc, in_maps, core_ids=list(range(NCORES)))
    out = np.empty((N, H), np.float32)
    for c in range(NCORES):
        out[node_lists[c]] = res.results[c]["out_hT"].T
    return out
